# revision 80
# baseline (speedup 1.0000x reference)
"""GAT (2-layer graph attention network) on 8 Trainium2 NeuronCores.

Row-parallel sharding: core c owns destination nodes [c*512, (c+1)*512).

Scores: e = leaky_relu(si + sj, 0.2); softmax over masked j is computed via
    exp(lrelu(si+sj)) / exp(si) = max(exp(sj), exp(-0.8*si) * exp(0.2*sj))
(the common factor exp(si) cancels in softmax normalization), so masked
unnormalized weights are  wm[j,i] = max(G[i]*F[j], E[j]) * A[j,i]  with
E=exp(sj), F=exp(0.2*sj), G=exp(-0.8*si).  On device that is ONE dual-op
tensor_scalar (mult then max, both per-partition scalars) per (head, j-tile)
plus one 4-head-wide tensor_tensor mask multiply with a step-0-broadcast
adjacency AP — no dense transcendentals on the hot path.  One head per
4-head group instead computes w = Exp(Prelu(si + sj)) on the otherwise-idle
ScalarE to offload VectorE.

Layer-1 si/sj projections (x @ (W1 a1)) are linear in the inputs, so they
are computed on the host in fp32 and fed in directly.  The layer-1 h values
(bf16, needed for ALL source nodes) are computed replicated on every core —
cheaper and better-overlapped than an all-gather.  Layer 2's x2 is genuinely
distributed, so h2 plus shard-computed exp(sj2)/exp(0.2*sj2) columns are
exchanged in ONE small fused AllGather; dummy TensorE matmuls bridge the
two PE-idle windows around it so the HAM clock-gate stays at 2.4 GHz.

Attention output and softmax denominators come from one TensorE matmul per
(head, j-tile) (ones column appended to h -> denominator row lands on psum
partition 64), in transposed [d, dest] orientation so layer 2's lhsT needs
no transpose.
"""

import numpy as np
import ml_dtypes

N, F, H, D, C = 4096, 512, 8, 64, 40
NCORES = 8
SH = N // NCORES      # 512 destination rows per core
JT = N // 128         # 32 j (source) tiles
KT = F // 128         # 4 k tiles over features
MT = SH // 128        # 4 m tiles over own rows
HCOL = D + 1          # 65 = h | ones
GRP = 4               # heads per mask group
ACT_K = (3,)          # group-local head indices computed on ScalarE
ALPHA = 0.2
NCH = 16              # x streaming chunks (JT // NCH j-tiles each; ~256KB per
                      # chunk so bulk transfers interleave finely with the
                      # small gating tiles in the DMA device FIFO)

_BUILT = {}
LAST_RESULTS = None


def _build():
    if "nc" in _BUILT:
        return _BUILT["nc"]
    import concourse.mybir as mybir
    import concourse.tile as tile
    from concourse import bacc

    f32 = mybir.dt.float32
    bf16 = mybir.dt.bfloat16
    AT = mybir.AluOpType
    ACT = mybir.ActivationFunctionType

    nc = bacc.Bacc("TRN2", num_devices=NCORES)

    xtb = nc.dram_tensor("xtb", [F, N], bf16, kind="ExternalInput")
    adjt = nc.dram_tensor("adjt", [N, SH], bf16, kind="ExternalInput")
    w1b = nc.dram_tensor("w1b", [F, H * D], bf16, kind="ExternalInput")
    e1f = nc.dram_tensor("e1f", [128, JT * H], f32, kind="ExternalInput")
    f1f = nc.dram_tensor("f1f", [128, JT * H], f32, kind="ExternalInput")
    sjb = nc.dram_tensor("sjb", [128, JT * H], f32, kind="ExternalInput")
    g1r = nc.dram_tensor("g1r", [H, SH], bf16, kind="ExternalInput")
    si1r = nc.dram_tensor("si1r", [H, SH], f32, kind="ExternalInput")
    w2f = nc.dram_tensor("w2f", [H * D, C], f32, kind="ExternalInput")
    ws2 = nc.dram_tensor("ws2", [H * D, 2], f32, kind="ExternalInput")
    outT = nc.dram_tensor("outT", [C, SH], f32, kind="ExternalOutput")

    with tile.TileContext(nc) as tc:
        with (
            tc.tile_pool(name="persist", bufs=1) as pp,
            tc.tile_pool(name="bcast", bufs=1) as pb,
            tc.tile_pool(name="xchunk", bufs=4) as px,
            tc.tile_pool(name="wsc", bufs=5) as pw,
            tc.tile_pool(name="epi", bufs=2) as pe,
            tc.tile_pool(name="psacc", bufs=6, space="PSUM") as ps_acc,
            tc.tile_pool(name="psep", bufs=2, space="PSUM") as ps_ep,
            tc.tile_pool(name="dram", bufs=1, space="DRAM") as pd,
        ):
            # ------------- score-side small inputs (host precomputed) -------------
            # DMA order is ramp-critical: the DMA device drains transfers in
            # enqueue order, so the small gating tiles (g1 rows -> broadcasts
            # -> first TS) and W1B/xc0 (first matmul) go first; bulk adjacency
            # and the rest of x stream behind them.
            G1R = pp.tile([1, H, SH], bf16, tag="g1r")
            nc.scalar.dma_start(G1R[:], g1r[:].rearrange("(o h) i -> o h i", o=1))
            E1 = pp.tile([128, JT, H], f32, tag="e1")
            nc.scalar.dma_start(E1[:], e1f[:].rearrange("p (jt h) -> p jt h", h=H))
            F1 = pp.tile([128, JT, H], f32, tag="f1")
            nc.scalar.dma_start(F1[:], f1f[:].rearrange("p (jt h) -> p jt h", h=H))
            W1B = pp.tile([128, KT, H * D], bf16, tag="w1b")
            nc.scalar.dma_start(W1B[:], w1b[:].rearrange("(kt p) c -> p kt c", p=128))
            xt_r0 = xtb[:].rearrange("(kt p) n -> p kt n", p=128)
            CW = N // NCH  # chunk width in source nodes
            xc0 = px.tile([128, KT, CW], bf16, tag="xc", name="xc_0")
            nc.scalar.dma_start(xc0[:], xt_r0[:, :, 0:CW])
            ADJ = pp.tile([128, JT, SH], bf16, tag="adj")
            adj_r = adjt[:].rearrange("(jt p) i -> p jt i", p=128)
            act_hs = [h for h in range(H) if (h % GRP) in ACT_K]
            SI1 = pp.tile([1, len(act_hs), SH], f32, tag="si1")
            for ai, h in enumerate(act_hs):
                nc.scalar.dma_start(SI1[:, ai, :],
                                    si1r[h:h + 1, :].rearrange("(o h) i -> o (h i)", o=1))
            SJB = pp.tile([128, JT, H], f32, tag="sjb")
            nc.scalar.dma_start(SJB[:], sjb[:].rearrange("p (jt h) -> p jt h", h=H))

            GB1, SIB = [], {}
            for h in range(H):
                gb = pb.tile([128, SH], bf16, tag=f"gb1_{h}", name=f"gb1_{h}")
                nc.gpsimd.partition_broadcast(gb[:], G1R[:, h, :])
                GB1.append(gb)
                if h == 1:
                    # first adjacency chunk: issued after the first broadcasts
                    # (which gate the score chain) but well before the first
                    # mask multiply needs it
                    nc.gpsimd.dma_start(ADJ[:, 0:2, :], adj_r[:, 0:2, :])
                if h >= GRP and (h % GRP) in ACT_K:
                    sb = pb.tile([128, SH], f32, tag=f"sib_{h}", name=f"sib_{h}")
                    nc.gpsimd.partition_broadcast(sb[:], SI1[:, act_hs.index(h), :])
                    SIB[h] = sb

            W2BS = pp.tile([64, H, C], f32, tag="w2bs")
            nc.scalar.dma_start(W2BS[:], w2f[:].rearrange("(h p) c -> p h c", p=64))
            WS2S = pp.tile([64, H, 2], f32, tag="ws2s")
            nc.scalar.dma_start(WS2S[:], ws2[:].rearrange("(h p) c -> p h c", p=64))

            grp = [list(range(NCORES))]
            NG = H // GRP  # head groups

            # ------- stage B (replicated h, streamed x) fused with group-A attention -------
            HBF = pp.tile([128, JT, H * HCOL], bf16, tag="hbf")
            ones_view = HBF[:].rearrange("p jt (h c) -> p jt h c", c=HCOL)[:, :, :, D:D + 1]
            nc.vector.memset(ones_view.opt(), 1.0)
            xt_r = xtb[:].rearrange("(kt p) n -> p kt n", p=128)
            CW = N // NCH  # chunk width in source nodes
            TPC = CW // 128

            X2T32 = [None] * H

            def emit_wm(g0, jt):
                # group A (fused with h production) keeps ScalarE free for the
                # h-copies: its k=3 head uses the normalized DVE-TS path; only
                # group B's k=3 head runs the Prelu+Exp path on ScalarE
                act_k = ACT_K if g0 == GRP else ()
                wsc = pw.tile([128, GRP, SH], bf16, tag="w", name=f"w_{g0}_{jt}")
                for k in range(GRP):
                    h = g0 + k
                    if k in act_k:
                        epre = ps_ep.tile([128, SH], f32, tag="epre", name=f"ep_{h}_{jt}")
                        nc.scalar.activation(epre[:], SIB[h][:], ACT.Prelu,
                                             bias=SJB[:, jt, h:h + 1], alpha=ALPHA)
                        nc.scalar.activation(wsc[:, k, :], epre[:], ACT.Exp)
                    else:
                        # k=2 GpSimd, k=1 GpSimd 3 of 4 j-tiles, k=0 DVE; the
                        # first 4 j-tiles stay on DVE (GpSimd's queue is still
                        # draining broadcasts during the ramp)
                        on_pool = jt >= 4 and ((k == 2) or (k == 1 and jt % 4 != 3))
                        eng = nc.gpsimd if on_pool else nc.vector
                        eng.tensor_scalar(wsc[:, k, :], GB1[h][:],
                                          F1[:, jt, h:h + 1], E1[:, jt, h:h + 1],
                                          AT.mult, AT.max)  # noqa: E501
                wm = pw.tile([128, GRP, SH], bf16, tag="wm", name=f"wm_{g0}_{jt}")
                a_rep = ADJ[:, jt, :].unsqueeze(1).to_broadcast([128, GRP, SH])
                nc.vector.tensor_tensor(wm[:], wsc[:], a_rep, AT.mult)
                return wm

            def emit_mms(g0, jt, psAs, wm):
                for k in range(GRP):
                    h = g0 + k
                    nc.tensor.matmul(psAs[k][:], HBF[:, jt, h * HCOL:(h + 1) * HCOL],
                                     wm[:, k, :], start=(jt == 0), stop=(jt == JT - 1))

            def emit_scores(g0, jt, psAs):
                emit_mms(g0, jt, psAs, emit_wm(g0, jt))

            def emit_epilogue(g0):
                # group B is column-chunked by m-tile: stage D's ph2(m) only
                # needs every head's m-th slice, so finer slices let the
                # gather start several us earlier
                mslices = ([slice(m * 128, (m + 1) * 128) for m in range(MT)]
                           if g0 == GRP else [slice(0, SH)])
                for k in range(GRP):
                    h = g0 + k
                    psA = psAs[k]
                    rc = pe.tile([1, SH], f32, tag="rc", name=f"rc_{h}", bufs=3)
                    nc.vector.reciprocal(rc[:], psA[D:D + 1, :])
                    rb = pe.tile([64, SH], f32, tag="rb", name=f"rb_{h}", bufs=3)
                    nc.gpsimd.partition_broadcast(rb[:], rc[:])
                    z = pe.tile([64, SH], f32, tag="z", name=f"z_{h}", bufs=3)
                    u = pe.tile([64, SH], f32, tag="u", name=f"u_{h}")
                    v = pe.tile([64, SH], f32, tag="v", name=f"v_{h}")
                    x2t = pp.tile([64, SH], f32, tag=f"x2t32_{h}", name=f"x2t_{h}")
                    for sl in mslices:
                        nc.vector.tensor_mul(z[:, sl], psA[0:D, sl], rb[:, sl])
                        nc.scalar.activation(u[:, sl], z[:, sl], ACT.Relu, scale=-1.0)
                        nc.scalar.activation(v[:, sl], u[:, sl], ACT.Exp, scale=-1.0)
                        nc.vector.scalar_tensor_tensor(x2t[:, sl], v[:, sl], -1.0,
                                                       z[:, sl], AT.add, AT.max)
                    X2T32[h] = x2t

            # LOOP1: stage-B h production + group-A (heads 0..3) attention per j-tile
            psAs = [ps_acc.tile([HCOL, SH], f32, tag="acc", name=f"psA_0_{k}")
                    for k in range(GRP)]
            # adjacency streams in 4-jt (512KB) chunks from the sync queue,
            # staying a few j-tiles ahead of consumption
            adj_sched = {0: (2, 6), 2: (6, 10), 5: (10, 14), 8: (14, 18),
                         11: (18, 22), 14: (22, 26), 17: (26, 30), 20: (30, 32)}
            xcs = {}
            xcs[0] = xc0

            def emit_h(jt):
                ch, t = divmod(jt, TPC)
                if t == 0 and ch > 0:
                    xc = px.tile([128, KT, CW], bf16, tag="xc", name=f"xc_{ch}")
                    nc.sync.dma_start(xc[:], xt_r[:, :, ch * CW:(ch + 1) * CW])
                    xcs[ch] = xc
                ph = ps_acc.tile([128, H * D], f32, tag="acc", name=f"ph_{jt}")
                for kt in range(KT):
                    nc.tensor.matmul(ph[:], xcs[ch][:, kt, t * 128:(t + 1) * 128],
                                     W1B[:, kt, :], start=(kt == 0), stop=(kt == KT - 1))
                dst = HBF[:, jt, :].rearrange("p (h c) -> p h c", c=HCOL)[:, :, 0:D]
                nc.scalar.copy(dst, ph[:].rearrange("p (h d) -> p h d", d=D))

            # software-pipeline the h production PROLOG tiles ahead: the
            # in-order PE queue would otherwise stall at the first attention
            # matmul (~8us, gated by broadcast->TS->mask) with no banked work
            PROLOG = 2
            for jt in range(PROLOG):
                emit_h(jt)
            for jt in range(JT):
                if jt in adj_sched:
                    lo, hi = adj_sched[jt]
                    nc.sync.dma_start(ADJ[:, lo:hi, :], adj_r[:, lo:hi, :])
                wm1 = emit_wm(0, jt)
                emit_mms(0, jt, psAs, wm1)
                # pipelined h production AFTER this j-tile's attention matmuls
                # so an x-chunk stall can't block them in the in-order PE queue
                if jt + PROLOG < JT:
                    emit_h(jt + PROLOG)
            emit_epilogue(0)

            # LOOP2: group-B (heads 4..7) attention
            psAs = [ps_acc.tile([HCOL, SH], f32, tag="acc", name=f"psA_4_{k}")
                    for k in range(GRP)]
            for jt in range(JT):
                emit_scores(GRP, jt, psAs)
            emit_epilogue(GRP)

            # bridge the group-B epilogue window (> HAM MID threshold) so
            # TensorE stays at 2.4 GHz into stage D
            warm1 = ps_acc.tile([HCOL, SH], f32, tag="acc", name="warm1")
            for wi in range(16):
                nc.tensor.matmul(warm1[:], HBF[:, wi % JT, 0:HCOL],
                                 HBF[:, wi % JT, 0:SH], start=True, stop=True)

            # ---------------- stage D: layer-2 shard compute (fp32) ----------------
            # single gather payload: [h2 as bf16 (40) | E2=exp(sj2) | F2=exp(0.2*sj2)]
            HB2S = pp.tile([128, MT, C + 2], bf16, tag="hb2s")
            for m in range(MT):
                ph2 = ps_acc.tile([128, C], f32, tag="acc", name=f"ph2_{m}")
                for h in range(H):
                    nc.tensor.matmul(ph2[:], X2T32[h][:, m * 128:(m + 1) * 128],
                                     W2BS[:, h, :], start=(h == 0), stop=(h == H - 1))
                nc.scalar.copy(HB2S[:, m, 0:C], ph2[:])
                psj2 = ps_acc.tile([128, 2], f32, tag="acc", name=f"psj2_{m}")
                for h in range(H):
                    nc.tensor.matmul(psj2[:], X2T32[h][:, m * 128:(m + 1) * 128],
                                     WS2S[:, h, :], start=(h == 0), stop=(h == H - 1))
                nc.scalar.activation(HB2S[:, m, C:C + 1], psj2[:, 0:1], ACT.Exp)
                nc.scalar.activation(HB2S[:, m, C + 1:C + 2], psj2[:, 0:1], ACT.Exp,
                                     scale=ALPHA)
            psi2 = ps_acc.tile([1, SH], f32, tag="acc")
            for h in range(H):
                nc.tensor.matmul(psi2[:], WS2S[:, h, 1:2], X2T32[h][:],
                                 start=(h == 0), stop=(h == H - 1))
            g2 = pe.tile([1, SH], bf16, tag="grow")
            nc.scalar.activation(g2[:], psi2[:], ACT.Exp, scale=-0.8)
            GB2 = pb.tile([128, SH], bf16, tag="gb2")
            nc.gpsimd.partition_broadcast(GB2[:], g2[:])

            hb2_bounce = pd.tile([SH, C + 2], bf16, tag="hb2_bounce")
            nc.sync.dma_start(hb2_bounce[:].rearrange("(m p) c -> p m c", p=128), HB2S[:])
            hb2f_d = nc.dram_tensor("hb2f_d", [N, C + 2], bf16, kind="Internal",
                                    addr_space="Shared")
            nc.gpsimd.collective_compute("AllGather", AT.bypass, replica_groups=grp,
                                         ins=[hb2_bounce.opt()], outs=[hb2f_d[:]])
            # (no warm matmuls here: they have no data dep on the collective,
            # so they fire during its early window and only delay stage E;
            # stage E itself is DVE/Pool-bound, so the PE p-state is moot)
            # one contiguous load of the gathered [h2|E2|F2] rows; the strided
            # sub-loads would each be descriptor-bound (~1.8us apiece), so
            # split on-chip with cheap 4x-mode DVE copies instead
            hb2f_r = hb2f_d[:].rearrange("(jt p) c -> p jt c", p=128)
            HB2A = pp.tile([128, JT, C + 2], bf16, tag="hb2a")
            nc.sync.dma_start(HB2A[:], hb2f_r[:])
            EF2 = pp.tile([128, JT, 2], f32, tag="ef2")
            nc.vector.tensor_copy(EF2[:], HB2A[:, :, C:C + 2])

            # [h2(40) | zero pad | ones at col 64] so the denominator row lands
            # on the 32-aligned psum partition 64
            HB2F = pp.tile([128, JT, HCOL], bf16, tag="hb2f")
            nc.vector.memset(HB2F[:], 0.0)
            nc.vector.tensor_copy(HB2F[:, :, 0:C], HB2A[:, :, 0:C])
            nc.vector.memset(HB2F[:, :, D:D + 1], 1.0)

            # ---------------- stage E: layer-2 attention (4-jt batches) ----------------
            ps2 = ps_acc.tile([HCOL, SH], f32, tag="acc")
            for jb in range(JT // 4):
                w2t = pw.tile([128, 4, SH], bf16, tag="w", name=f"w2t_{jb}")
                for t in range(4):
                    jt = jb * 4 + t
                    eng2 = nc.gpsimd if t >= 2 else nc.vector
                    eng2.tensor_scalar(w2t[:, t, :], GB2[:],
                                       EF2[:, jt, 1:2], EF2[:, jt, 0:1],
                                       AT.mult, AT.max)
                wm2 = pw.tile([128, 4, SH], bf16, tag="wm", name=f"wm2_{jb}")
                nc.vector.tensor_tensor(wm2[:], w2t[:], ADJ[:, jb * 4:(jb + 1) * 4, :], AT.mult)
                for t in range(4):
                    jt = jb * 4 + t
                    nc.tensor.matmul(ps2[:], HB2F[:, jt, :], wm2[:, t, :],
                                     start=(jt == 0), stop=(jt == JT - 1))
            rc2 = pe.tile([1, SH], f32, tag="rc", bufs=3)
            nc.vector.reciprocal(rc2[:], ps2[D:D + 1, :])
            rb2 = pe.tile([64, SH], f32, tag="rb", bufs=3)
            nc.gpsimd.partition_broadcast(rb2[:], rc2[:])
            OT = pe.tile([64, SH], f32, tag="z", bufs=3)
            nc.vector.tensor_mul(OT[:], ps2[0:64, :], rb2[:])
            nc.sync.dma_start(outT[:], OT[0:C, :])

    nc.compile()
    _BUILT["nc"] = nc
    return nc


def kernel(x, adj, W1, a1_src, a1_dst, W2, a2_src, a2_dst):
    global LAST_RESULTS
    from concourse.bass_utils import run_bass_kernel_spmd

    bf = ml_dtypes.bfloat16
    x = np.asarray(x, np.float32)
    adj = np.asarray(adj)
    W1 = np.asarray(W1, np.float32)
    W2 = np.asarray(W2, np.float32)
    a1_src = np.asarray(a1_src, np.float32)
    a1_dst = np.asarray(a1_dst, np.float32)
    a2_src = np.asarray(a2_src, np.float32)
    a2_dst = np.asarray(a2_dst, np.float32)

    xt = x.T.astype(bf)                                 # [F, N] (astype -> contiguous)
    adjt = adj.T.astype(bf)                             # [N(j), N(i)]
    w1b = np.ascontiguousarray(W1.transpose(1, 0, 2).reshape(F, H * D)).astype(bf)
    w2f = W2.astype(np.float32)
    ws2 = np.ascontiguousarray(np.stack([W2 @ a2_src, W2 @ a2_dst], axis=1)).astype(np.float32)

    # host-side linear projections for layer-1 scores (exact fp32)
    sj = x @ np.einsum("hfd,hd->fh", W1, a1_src)        # [N, H]
    si = x @ np.einsum("hfd,hd->fh", W1, a1_dst)        # [N, H]
    dev = lambda a: np.ascontiguousarray(
        a.reshape(JT, 128, H).transpose(1, 0, 2).reshape(128, JT * H)).astype(np.float32)
    E = np.exp(sj)
    Fj = np.exp(ALPHA * sj)

    e1f, f1f, sjb = dev(E), dev(Fj), dev(sj)
    nc = _build()
    in_maps = []
    for c in range(NCORES):
        lo, hi = c * SH, (c + 1) * SH
        si_own = si[lo:hi, :]                           # [SH, H]
        in_maps.append(dict(
            xtb=xt,
            adjt=np.ascontiguousarray(adjt[:, lo:hi]),
            w1b=w1b, w2f=w2f, ws2=ws2,
            e1f=e1f, f1f=f1f, sjb=sjb,
            g1r=np.ascontiguousarray(np.exp(-0.8 * si_own.T)).astype(bf),
            si1r=np.ascontiguousarray(si_own.T).astype(np.float32),
        ))
    res = run_bass_kernel_spmd(nc, in_maps, core_ids=list(range(NCORES)))
    LAST_RESULTS = res
    out = np.concatenate([res.results[c]["outT"].T for c in range(NCORES)], axis=0)
    return np.ascontiguousarray(out.astype(np.float32))



# revision 81
# speedup vs baseline: 1.0114x; 1.0114x over previous
"""GAT (2-layer graph attention network) on 8 Trainium2 NeuronCores.

Row-parallel sharding: core c owns destination nodes [c*512, (c+1)*512).

Scores: e = leaky_relu(si + sj, 0.2); softmax over masked j is computed via
    exp(lrelu(si+sj)) / exp(si) = max(exp(sj), exp(-0.8*si) * exp(0.2*sj))
(the common factor exp(si) cancels in softmax normalization), so masked
unnormalized weights are  wm[j,i] = max(G[i]*F[j], E[j]) * A[j,i]  with
E=exp(sj), F=exp(0.2*sj), G=exp(-0.8*si).  On device that is ONE dual-op
tensor_scalar (mult then max, both per-partition scalars) per (head, j-tile)
plus one 4-head-wide tensor_tensor mask multiply with a step-0-broadcast
adjacency AP — no dense transcendentals on the hot path.  One head per
4-head group instead computes w = Exp(Prelu(si + sj)) on the otherwise-idle
ScalarE to offload VectorE.

Layer-1 si/sj projections (x @ (W1 a1)) are linear in the inputs, so they
are computed on the host in fp32 and fed in directly.  The layer-1 h values
(bf16, needed for ALL source nodes) are computed replicated on every core —
cheaper and better-overlapped than an all-gather.  Layer 2's x2 is genuinely
distributed, so h2 plus shard-computed exp(sj2)/exp(0.2*sj2) columns are
exchanged in ONE small fused AllGather; dummy TensorE matmuls bridge the
two PE-idle windows around it so the HAM clock-gate stays at 2.4 GHz.

Attention output and softmax denominators come from one TensorE matmul per
(head, j-tile) (ones column appended to h -> denominator row lands on psum
partition 64), in transposed [d, dest] orientation so layer 2's lhsT needs
no transpose.
"""

import numpy as np
import ml_dtypes

N, F, H, D, C = 4096, 512, 8, 64, 40
NCORES = 8
SH = N // NCORES      # 512 destination rows per core
JT = N // 128         # 32 j (source) tiles
KT = F // 128         # 4 k tiles over features
MT = SH // 128        # 4 m tiles over own rows
HCOL = D + 1          # 65 = h | ones
GRP = 4               # heads per mask group
ACT_K = (3,)          # group-local head indices computed on ScalarE
ALPHA = 0.2
NCH = 16              # x streaming chunks (JT // NCH j-tiles each; ~256KB per
                      # chunk so bulk transfers interleave finely with the
                      # small gating tiles in the DMA device FIFO)

_BUILT = {}
LAST_RESULTS = None


def _build():
    if "nc" in _BUILT:
        return _BUILT["nc"]
    import concourse.mybir as mybir
    import concourse.tile as tile
    from concourse import bacc

    f32 = mybir.dt.float32
    bf16 = mybir.dt.bfloat16
    AT = mybir.AluOpType
    ACT = mybir.ActivationFunctionType

    nc = bacc.Bacc("TRN2", num_devices=NCORES)

    xtb = nc.dram_tensor("xtb", [F, N], bf16, kind="ExternalInput")
    adjt = nc.dram_tensor("adjt", [N, SH], bf16, kind="ExternalInput")
    w1b = nc.dram_tensor("w1b", [F, H * D], bf16, kind="ExternalInput")
    e1f = nc.dram_tensor("e1f", [128, JT * H], f32, kind="ExternalInput")
    f1f = nc.dram_tensor("f1f", [128, JT * H], f32, kind="ExternalInput")
    sjb = nc.dram_tensor("sjb", [128, JT * H], f32, kind="ExternalInput")
    g1r = nc.dram_tensor("g1r", [H, SH], bf16, kind="ExternalInput")
    si1r = nc.dram_tensor("si1r", [H, SH], f32, kind="ExternalInput")
    w2f = nc.dram_tensor("w2f", [H * D, C], f32, kind="ExternalInput")
    ws2 = nc.dram_tensor("ws2", [H * D, 2], f32, kind="ExternalInput")
    outT = nc.dram_tensor("outT", [C, SH], f32, kind="ExternalOutput")

    with tile.TileContext(nc) as tc:
        with (
            tc.tile_pool(name="persist", bufs=1) as pp,
            tc.tile_pool(name="bcast", bufs=1) as pb,
            tc.tile_pool(name="xchunk", bufs=4) as px,
            tc.tile_pool(name="wsc", bufs=5) as pw,
            tc.tile_pool(name="epi", bufs=2) as pe,
            tc.tile_pool(name="psacc", bufs=6, space="PSUM") as ps_acc,
            tc.tile_pool(name="psep", bufs=2, space="PSUM") as ps_ep,
            tc.tile_pool(name="dram", bufs=1, space="DRAM") as pd,
        ):
            # ------------- score-side small inputs (host precomputed) -------------
            # DMA order is ramp-critical: the DMA device drains transfers in
            # enqueue order, so the small gating tiles (g1 rows -> broadcasts
            # -> first TS) and W1B/xc0 (first matmul) go first; bulk adjacency
            # and the rest of x stream behind them.
            G1R = pp.tile([1, H, SH], bf16, tag="g1r")
            nc.scalar.dma_start(G1R[:], g1r[:].rearrange("(o h) i -> o h i", o=1))
            E1 = pp.tile([128, JT, H], f32, tag="e1")
            nc.scalar.dma_start(E1[:], e1f[:].rearrange("p (jt h) -> p jt h", h=H))
            F1 = pp.tile([128, JT, H], f32, tag="f1")
            nc.scalar.dma_start(F1[:], f1f[:].rearrange("p (jt h) -> p jt h", h=H))
            W1B = pp.tile([128, KT, H * D], bf16, tag="w1b")
            nc.scalar.dma_start(W1B[:], w1b[:].rearrange("(kt p) c -> p kt c", p=128))
            xt_r0 = xtb[:].rearrange("(kt p) n -> p kt n", p=128)
            CW = N // NCH  # chunk width in source nodes
            xc0 = px.tile([128, KT, CW], bf16, tag="xc", name="xc_0")
            nc.scalar.dma_start(xc0[:], xt_r0[:, :, 0:CW])
            ADJ = pp.tile([128, JT, SH], bf16, tag="adj")
            adj_r = adjt[:].rearrange("(jt p) i -> p jt i", p=128)
            act_hs = [h for h in range(H) if (h % GRP) in ACT_K]
            SI1 = pp.tile([1, len(act_hs), SH], f32, tag="si1")
            for ai, h in enumerate(act_hs):
                nc.scalar.dma_start(SI1[:, ai, :],
                                    si1r[h:h + 1, :].rearrange("(o h) i -> o (h i)", o=1))
            SJB = pp.tile([128, JT, H], f32, tag="sjb")
            nc.scalar.dma_start(SJB[:], sjb[:].rearrange("p (jt h) -> p jt h", h=H))

            GB1, SIB = [], {}
            for h in range(H):
                gb = pb.tile([128, SH], bf16, tag=f"gb1_{h}", name=f"gb1_{h}")
                nc.gpsimd.partition_broadcast(gb[:], G1R[:, h, :])
                GB1.append(gb)
                if h == 1:
                    # first adjacency chunk: issued after the first broadcasts
                    # (which gate the score chain) but well before the first
                    # mask multiply needs it
                    nc.gpsimd.dma_start(ADJ[:, 0:2, :], adj_r[:, 0:2, :])
                if h >= GRP and (h % GRP) in ACT_K:
                    sb = pb.tile([128, SH], f32, tag=f"sib_{h}", name=f"sib_{h}")
                    nc.gpsimd.partition_broadcast(sb[:], SI1[:, act_hs.index(h), :])
                    SIB[h] = sb

            W2BS = pp.tile([64, H, C], f32, tag="w2bs")
            nc.scalar.dma_start(W2BS[:], w2f[:].rearrange("(h p) c -> p h c", p=64))
            WS2S = pp.tile([64, H, 2], f32, tag="ws2s")
            nc.scalar.dma_start(WS2S[:], ws2[:].rearrange("(h p) c -> p h c", p=64))

            grp = [list(range(NCORES))]
            NG = H // GRP  # head groups

            # ------- stage B (replicated h, streamed x) fused with group-A attention -------
            HBF = pp.tile([128, JT, H * HCOL], bf16, tag="hbf")
            ones_view = HBF[:].rearrange("p jt (h c) -> p jt h c", c=HCOL)[:, :, :, D:D + 1]
            nc.vector.memset(ones_view.opt(), 1.0)
            xt_r = xtb[:].rearrange("(kt p) n -> p kt n", p=128)
            CW = N // NCH  # chunk width in source nodes
            TPC = CW // 128

            X2T32 = [None] * H

            def emit_wm(g0, jt):
                # group A (fused with h production) keeps ScalarE free for the
                # h-copies: its k=3 head uses the normalized DVE-TS path; only
                # group B's k=3 head runs the Prelu+Exp path on ScalarE
                act_k = ACT_K if g0 == GRP else ()
                wsc = pw.tile([128, GRP, SH], bf16, tag="w", name=f"w_{g0}_{jt}")
                for k in range(GRP):
                    h = g0 + k
                    if k in act_k:
                        epre = ps_ep.tile([128, SH], f32, tag="epre", name=f"ep_{h}_{jt}")
                        nc.scalar.activation(epre[:], SIB[h][:], ACT.Prelu,
                                             bias=SJB[:, jt, h:h + 1], alpha=ALPHA)
                        nc.scalar.activation(wsc[:, k, :], epre[:], ACT.Exp)
                    else:
                        # k=2 GpSimd, k=1 GpSimd 3 of 4 j-tiles, k=0 DVE; the
                        # first 4 j-tiles stay on DVE (GpSimd's queue is still
                        # draining broadcasts during the ramp)
                        on_pool = jt >= 4 and ((k == 2) or (k == 1 and jt % 4 != 3))
                        eng = nc.gpsimd if on_pool else nc.vector
                        eng.tensor_scalar(wsc[:, k, :], GB1[h][:],
                                          F1[:, jt, h:h + 1], E1[:, jt, h:h + 1],
                                          AT.mult, AT.max)  # noqa: E501
                wm = pw.tile([128, GRP, SH], bf16, tag="wm", name=f"wm_{g0}_{jt}")
                a_rep = ADJ[:, jt, :].unsqueeze(1).to_broadcast([128, GRP, SH])
                nc.vector.tensor_tensor(wm[:], wsc[:], a_rep, AT.mult)
                return wm

            def emit_mms(g0, jt, psAs, wm):
                for k in range(GRP):
                    h = g0 + k
                    nc.tensor.matmul(psAs[k][:], HBF[:, jt, h * HCOL:(h + 1) * HCOL],
                                     wm[:, k, :], start=(jt == 0), stop=(jt == JT - 1))

            def emit_scores(g0, jt, psAs):
                emit_mms(g0, jt, psAs, emit_wm(g0, jt))

            def emit_epilogue(g0):
                # (column-chunking group B by m-tile to start stage D earlier
                # was tried and regressed: the 4x instruction count costs more
                # than the earlier gather start saves)
                mslices = [slice(0, SH)]
                for k in range(GRP):
                    h = g0 + k
                    psA = psAs[k]
                    rc = pe.tile([1, SH], f32, tag="rc", name=f"rc_{h}", bufs=3)
                    nc.vector.reciprocal(rc[:], psA[D:D + 1, :])
                    rb = pe.tile([64, SH], f32, tag="rb", name=f"rb_{h}", bufs=3)
                    nc.gpsimd.partition_broadcast(rb[:], rc[:])
                    z = pe.tile([64, SH], f32, tag="z", name=f"z_{h}", bufs=3)
                    u = pe.tile([64, SH], f32, tag="u", name=f"u_{h}")
                    v = pe.tile([64, SH], f32, tag="v", name=f"v_{h}")
                    x2t = pp.tile([64, SH], f32, tag=f"x2t32_{h}", name=f"x2t_{h}")
                    for sl in mslices:
                        nc.vector.tensor_mul(z[:, sl], psA[0:D, sl], rb[:, sl])
                        nc.scalar.activation(u[:, sl], z[:, sl], ACT.Relu, scale=-1.0)
                        nc.scalar.activation(v[:, sl], u[:, sl], ACT.Exp, scale=-1.0)
                        nc.vector.scalar_tensor_tensor(x2t[:, sl], v[:, sl], -1.0,
                                                       z[:, sl], AT.add, AT.max)
                    X2T32[h] = x2t

            # LOOP1: stage-B h production + group-A (heads 0..3) attention per j-tile
            psAs = [ps_acc.tile([HCOL, SH], f32, tag="acc", name=f"psA_0_{k}")
                    for k in range(GRP)]
            # adjacency streams in 4-jt (512KB) chunks from the sync queue,
            # staying a few j-tiles ahead of consumption
            adj_sched = {0: (2, 6), 2: (6, 10), 5: (10, 14), 8: (14, 18),
                         11: (18, 22), 14: (22, 26), 17: (26, 30), 20: (30, 32)}
            xcs = {}
            xcs[0] = xc0

            def emit_h(jt):
                ch, t = divmod(jt, TPC)
                if t == 0 and ch > 0:
                    xc = px.tile([128, KT, CW], bf16, tag="xc", name=f"xc_{ch}")
                    nc.sync.dma_start(xc[:], xt_r[:, :, ch * CW:(ch + 1) * CW])
                    xcs[ch] = xc
                ph = ps_acc.tile([128, H * D], f32, tag="acc", name=f"ph_{jt}")
                for kt in range(KT):
                    nc.tensor.matmul(ph[:], xcs[ch][:, kt, t * 128:(t + 1) * 128],
                                     W1B[:, kt, :], start=(kt == 0), stop=(kt == KT - 1))
                dst = HBF[:, jt, :].rearrange("p (h c) -> p h c", c=HCOL)[:, :, 0:D]
                nc.scalar.copy(dst, ph[:].rearrange("p (h d) -> p h d", d=D))

            # software-pipeline the h production PROLOG tiles ahead: the
            # in-order PE queue would otherwise stall at the first attention
            # matmul (~8us, gated by broadcast->TS->mask) with no banked work
            PROLOG = 2
            for jt in range(PROLOG):
                emit_h(jt)
            for jt in range(JT):
                if jt in adj_sched:
                    lo, hi = adj_sched[jt]
                    nc.sync.dma_start(ADJ[:, lo:hi, :], adj_r[:, lo:hi, :])
                wm1 = emit_wm(0, jt)
                emit_mms(0, jt, psAs, wm1)
                # pipelined h production AFTER this j-tile's attention matmuls
                # so an x-chunk stall can't block them in the in-order PE queue
                if jt + PROLOG < JT:
                    emit_h(jt + PROLOG)
            emit_epilogue(0)

            # LOOP2: group-B (heads 4..7) attention
            psAs = [ps_acc.tile([HCOL, SH], f32, tag="acc", name=f"psA_4_{k}")
                    for k in range(GRP)]
            for jt in range(JT):
                emit_scores(GRP, jt, psAs)
            emit_epilogue(GRP)

            # bridge the group-B epilogue window (> HAM MID threshold) so
            # TensorE stays at 2.4 GHz into stage D
            warm1 = ps_acc.tile([HCOL, SH], f32, tag="acc", name="warm1")
            for wi in range(16):
                nc.tensor.matmul(warm1[:], HBF[:, wi % JT, 0:HCOL],
                                 HBF[:, wi % JT, 0:SH], start=True, stop=True)

            # ---------------- stage D: layer-2 shard compute (fp32) ----------------
            # single gather payload: [h2 as bf16 (40) | E2=exp(sj2) | F2=exp(0.2*sj2)]
            HB2S = pp.tile([128, MT, C + 2], bf16, tag="hb2s")
            for m in range(MT):
                ph2 = ps_acc.tile([128, C], f32, tag="acc", name=f"ph2_{m}")
                for h in range(H):
                    nc.tensor.matmul(ph2[:], X2T32[h][:, m * 128:(m + 1) * 128],
                                     W2BS[:, h, :], start=(h == 0), stop=(h == H - 1))
                nc.scalar.copy(HB2S[:, m, 0:C], ph2[:])
                psj2 = ps_acc.tile([128, 2], f32, tag="acc", name=f"psj2_{m}")
                for h in range(H):
                    nc.tensor.matmul(psj2[:], X2T32[h][:, m * 128:(m + 1) * 128],
                                     WS2S[:, h, :], start=(h == 0), stop=(h == H - 1))
                nc.scalar.activation(HB2S[:, m, C:C + 1], psj2[:, 0:1], ACT.Exp)
                nc.scalar.activation(HB2S[:, m, C + 1:C + 2], psj2[:, 0:1], ACT.Exp,
                                     scale=ALPHA)
            psi2 = ps_acc.tile([1, SH], f32, tag="acc")
            for h in range(H):
                nc.tensor.matmul(psi2[:], WS2S[:, h, 1:2], X2T32[h][:],
                                 start=(h == 0), stop=(h == H - 1))
            g2 = pe.tile([1, SH], bf16, tag="grow")
            nc.scalar.activation(g2[:], psi2[:], ACT.Exp, scale=-0.8)
            GB2 = pb.tile([128, SH], bf16, tag="gb2")
            nc.gpsimd.partition_broadcast(GB2[:], g2[:])

            hb2_bounce = pd.tile([SH, C + 2], bf16, tag="hb2_bounce")
            nc.sync.dma_start(hb2_bounce[:].rearrange("(m p) c -> p m c", p=128), HB2S[:])
            hb2f_d = nc.dram_tensor("hb2f_d", [N, C + 2], bf16, kind="Internal",
                                    addr_space="Shared")
            nc.gpsimd.collective_compute("AllGather", AT.bypass, replica_groups=grp,
                                         ins=[hb2_bounce.opt()], outs=[hb2f_d[:]])
            # (no warm matmuls here: they have no data dep on the collective,
            # so they fire during its early window and only delay stage E;
            # stage E itself is DVE/Pool-bound, so the PE p-state is moot)
            # one contiguous load of the gathered [h2|E2|F2] rows; the strided
            # sub-loads would each be descriptor-bound (~1.8us apiece), so
            # split on-chip with cheap 4x-mode DVE copies instead
            hb2f_r = hb2f_d[:].rearrange("(jt p) c -> p jt c", p=128)
            HB2A = pp.tile([128, JT, C + 2], bf16, tag="hb2a")
            nc.sync.dma_start(HB2A[:], hb2f_r[:])
            EF2 = pp.tile([128, JT, 2], f32, tag="ef2")
            nc.vector.tensor_copy(EF2[:], HB2A[:, :, C:C + 2])

            # [h2(40) | zero pad | ones at col 64] so the denominator row lands
            # on the 32-aligned psum partition 64
            HB2F = pp.tile([128, JT, HCOL], bf16, tag="hb2f")
            nc.vector.memset(HB2F[:], 0.0)
            nc.vector.tensor_copy(HB2F[:, :, 0:C], HB2A[:, :, 0:C])
            nc.vector.memset(HB2F[:, :, D:D + 1], 1.0)

            # ---------------- stage E: layer-2 attention (4-jt batches) ----------------
            ps2 = ps_acc.tile([HCOL, SH], f32, tag="acc")
            for jb in range(JT // 4):
                w2t = pw.tile([128, 4, SH], bf16, tag="w", name=f"w2t_{jb}")
                for t in range(4):
                    jt = jb * 4 + t
                    eng2 = nc.gpsimd if t >= 2 else nc.vector
                    eng2.tensor_scalar(w2t[:, t, :], GB2[:],
                                       EF2[:, jt, 1:2], EF2[:, jt, 0:1],
                                       AT.mult, AT.max)
                wm2 = pw.tile([128, 4, SH], bf16, tag="wm", name=f"wm2_{jb}")
                nc.vector.tensor_tensor(wm2[:], w2t[:], ADJ[:, jb * 4:(jb + 1) * 4, :], AT.mult)
                for t in range(4):
                    jt = jb * 4 + t
                    nc.tensor.matmul(ps2[:], HB2F[:, jt, :], wm2[:, t, :],
                                     start=(jt == 0), stop=(jt == JT - 1))
            rc2 = pe.tile([1, SH], f32, tag="rc", bufs=3)
            nc.vector.reciprocal(rc2[:], ps2[D:D + 1, :])
            rb2 = pe.tile([64, SH], f32, tag="rb", bufs=3)
            nc.gpsimd.partition_broadcast(rb2[:], rc2[:])
            OT = pe.tile([64, SH], f32, tag="z", bufs=3)
            nc.vector.tensor_mul(OT[:], ps2[0:64, :], rb2[:])
            nc.sync.dma_start(outT[:], OT[0:C, :])

    nc.compile()
    _BUILT["nc"] = nc
    return nc


def kernel(x, adj, W1, a1_src, a1_dst, W2, a2_src, a2_dst):
    global LAST_RESULTS
    from concourse.bass_utils import run_bass_kernel_spmd

    bf = ml_dtypes.bfloat16
    x = np.asarray(x, np.float32)
    adj = np.asarray(adj)
    W1 = np.asarray(W1, np.float32)
    W2 = np.asarray(W2, np.float32)
    a1_src = np.asarray(a1_src, np.float32)
    a1_dst = np.asarray(a1_dst, np.float32)
    a2_src = np.asarray(a2_src, np.float32)
    a2_dst = np.asarray(a2_dst, np.float32)

    xt = x.T.astype(bf)                                 # [F, N] (astype -> contiguous)
    adjt = adj.T.astype(bf)                             # [N(j), N(i)]
    w1b = np.ascontiguousarray(W1.transpose(1, 0, 2).reshape(F, H * D)).astype(bf)
    w2f = W2.astype(np.float32)
    ws2 = np.ascontiguousarray(np.stack([W2 @ a2_src, W2 @ a2_dst], axis=1)).astype(np.float32)

    # host-side linear projections for layer-1 scores (exact fp32)
    sj = x @ np.einsum("hfd,hd->fh", W1, a1_src)        # [N, H]
    si = x @ np.einsum("hfd,hd->fh", W1, a1_dst)        # [N, H]
    dev = lambda a: np.ascontiguousarray(
        a.reshape(JT, 128, H).transpose(1, 0, 2).reshape(128, JT * H)).astype(np.float32)
    E = np.exp(sj)
    Fj = np.exp(ALPHA * sj)

    e1f, f1f, sjb = dev(E), dev(Fj), dev(sj)
    nc = _build()
    in_maps = []
    for c in range(NCORES):
        lo, hi = c * SH, (c + 1) * SH
        si_own = si[lo:hi, :]                           # [SH, H]
        in_maps.append(dict(
            xtb=xt,
            adjt=np.ascontiguousarray(adjt[:, lo:hi]),
            w1b=w1b, w2f=w2f, ws2=ws2,
            e1f=e1f, f1f=f1f, sjb=sjb,
            g1r=np.ascontiguousarray(np.exp(-0.8 * si_own.T)).astype(bf),
            si1r=np.ascontiguousarray(si_own.T).astype(np.float32),
        ))
    res = run_bass_kernel_spmd(nc, in_maps, core_ids=list(range(NCORES)))
    LAST_RESULTS = res
    out = np.concatenate([res.results[c]["outT"].T for c in range(NCORES)], axis=0)
    return np.ascontiguousarray(out.astype(np.float32))



# revision 83
# speedup vs baseline: 1.0211x; 1.0095x over previous
"""GAT (2-layer graph attention network) on 8 Trainium2 NeuronCores.

Row-parallel sharding: core c owns destination nodes [c*512, (c+1)*512).

Scores: e = leaky_relu(si + sj, 0.2); softmax over masked j is computed via
    exp(lrelu(si+sj)) / exp(si) = max(exp(sj), exp(-0.8*si) * exp(0.2*sj))
(the common factor exp(si) cancels in softmax normalization), so masked
unnormalized weights are  wm[j,i] = max(G[i]*F[j], E[j]) * A[j,i]  with
E=exp(sj), F=exp(0.2*sj), G=exp(-0.8*si).  On device that is ONE dual-op
tensor_scalar (mult then max, both per-partition scalars) per (head, j-tile)
plus one 4-head-wide tensor_tensor mask multiply with a step-0-broadcast
adjacency AP — no dense transcendentals on the hot path.  One head per
4-head group instead computes w = Exp(Prelu(si + sj)) on the otherwise-idle
ScalarE to offload VectorE.

Layer-1 si/sj projections (x @ (W1 a1)) are linear in the inputs, so they
are computed on the host in fp32 and fed in directly.  The layer-1 h values
(bf16, needed for ALL source nodes) are computed replicated on every core —
cheaper and better-overlapped than an all-gather.  Layer 2's x2 is genuinely
distributed, so h2 plus shard-computed exp(sj2)/exp(0.2*sj2) columns are
exchanged in ONE small fused AllGather; dummy TensorE matmuls bridge the
two PE-idle windows around it so the HAM clock-gate stays at 2.4 GHz.

Attention output and softmax denominators come from one TensorE matmul per
(head, j-tile) (ones column appended to h -> denominator row lands on psum
partition 64), in transposed [d, dest] orientation so layer 2's lhsT needs
no transpose.
"""

import numpy as np
import ml_dtypes

N, F, H, D, C = 4096, 512, 8, 64, 40
NCORES = 8
SH = N // NCORES      # 512 destination rows per core
JT = N // 128         # 32 j (source) tiles
KT = F // 128         # 4 k tiles over features
MT = SH // 128        # 4 m tiles over own rows
HCOL = D + 1          # 65 = h | ones
GRP = 4               # heads per mask group
ACT_K = (3,)          # group-local head indices computed on ScalarE
ALPHA = 0.2
NCH = 16              # x streaming chunks (JT // NCH j-tiles each; ~256KB per
                      # chunk so bulk transfers interleave finely with the
                      # small gating tiles in the DMA device FIFO)

_BUILT = {}
LAST_RESULTS = None


def _build():
    if "nc" in _BUILT:
        return _BUILT["nc"]
    import concourse.mybir as mybir
    import concourse.tile as tile
    from concourse import bacc

    f32 = mybir.dt.float32
    bf16 = mybir.dt.bfloat16
    AT = mybir.AluOpType
    ACT = mybir.ActivationFunctionType

    nc = bacc.Bacc("TRN2", num_devices=NCORES)

    xtb = nc.dram_tensor("xtb", [F, N], bf16, kind="ExternalInput")
    adjt = nc.dram_tensor("adjt", [N, SH], bf16, kind="ExternalInput")
    w1b = nc.dram_tensor("w1b", [F, H * D], bf16, kind="ExternalInput")
    e1f = nc.dram_tensor("e1f", [128, JT * H], f32, kind="ExternalInput")
    f1f = nc.dram_tensor("f1f", [128, JT * H], f32, kind="ExternalInput")
    sjb = nc.dram_tensor("sjb", [128, JT * H], f32, kind="ExternalInput")
    g1r = nc.dram_tensor("g1r", [H, SH], bf16, kind="ExternalInput")
    si1r = nc.dram_tensor("si1r", [H, SH], f32, kind="ExternalInput")
    w2f = nc.dram_tensor("w2f", [H * D, C], f32, kind="ExternalInput")
    ws2 = nc.dram_tensor("ws2", [H * D, 2], f32, kind="ExternalInput")
    outT = nc.dram_tensor("outT", [C, SH], f32, kind="ExternalOutput")

    with tile.TileContext(nc) as tc:
        with (
            tc.tile_pool(name="persist", bufs=1) as pp,
            tc.tile_pool(name="bcast", bufs=1) as pb,
            tc.tile_pool(name="xchunk", bufs=4) as px,
            tc.tile_pool(name="wsc", bufs=5) as pw,
            tc.tile_pool(name="epi", bufs=2) as pe,
            tc.tile_pool(name="psacc", bufs=6, space="PSUM") as ps_acc,
            tc.tile_pool(name="psep", bufs=2, space="PSUM") as ps_ep,
            tc.tile_pool(name="dram", bufs=1, space="DRAM") as pd,
        ):
            # ------------- score-side small inputs (host precomputed) -------------
            # DMA order is ramp-critical: the DMA device drains transfers in
            # enqueue order, so the small gating tiles (g1 rows -> broadcasts
            # -> first TS) and W1B/xc0 (first matmul) go first; bulk adjacency
            # and the rest of x stream behind them.
            G1R = pp.tile([1, H, SH], bf16, tag="g1r")
            nc.scalar.dma_start(G1R[:], g1r[:].rearrange("(o h) i -> o h i", o=1))
            ADJ = pp.tile([128, JT, SH], bf16, tag="adj")
            adj_r = adjt[:].rearrange("(jt p) i -> p jt i", p=128)
            nc.scalar.dma_start(ADJ[:, 0:2, :], adj_r[:, 0:2, :])
            E1 = pp.tile([128, JT, H], f32, tag="e1")
            nc.scalar.dma_start(E1[:], e1f[:].rearrange("p (jt h) -> p jt h", h=H))
            F1 = pp.tile([128, JT, H], f32, tag="f1")
            nc.scalar.dma_start(F1[:], f1f[:].rearrange("p (jt h) -> p jt h", h=H))
            # W1B/xc0 (the TensorE gates) issue from the gpsimd queue head so
            # they hit the DMA device FIFO ahead of the sync-queue bulk flood
            W1B = pp.tile([128, KT, H * D], bf16, tag="w1b")
            nc.gpsimd.dma_start(W1B[:], w1b[:].rearrange("(kt p) c -> p kt c", p=128))
            xt_r0 = xtb[:].rearrange("(kt p) n -> p kt n", p=128)
            CW = N // NCH  # chunk width in source nodes
            xc0 = px.tile([128, KT, CW], bf16, tag="xc", name="xc_0")
            nc.gpsimd.dma_start(xc0[:], xt_r0[:, :, 0:CW])
            act_hs = [h for h in range(H) if (h % GRP) in ACT_K]
            SI1 = pp.tile([1, len(act_hs), SH], f32, tag="si1")
            for ai, h in enumerate(act_hs):
                nc.scalar.dma_start(SI1[:, ai, :],
                                    si1r[h:h + 1, :].rearrange("(o h) i -> o (h i)", o=1))
            SJB = pp.tile([128, JT, H], f32, tag="sjb")
            nc.scalar.dma_start(SJB[:], sjb[:].rearrange("p (jt h) -> p jt h", h=H))

            GB1, SIB = [], {}
            for h in range(H):
                gb = pb.tile([128, SH], bf16, tag=f"gb1_{h}", name=f"gb1_{h}")
                nc.gpsimd.partition_broadcast(gb[:], G1R[:, h, :])
                GB1.append(gb)
                if h >= GRP and (h % GRP) in ACT_K:
                    sb = pb.tile([128, SH], f32, tag=f"sib_{h}", name=f"sib_{h}")
                    nc.gpsimd.partition_broadcast(sb[:], SI1[:, act_hs.index(h), :])
                    SIB[h] = sb

            W2BS = pp.tile([64, H, C], f32, tag="w2bs")
            nc.scalar.dma_start(W2BS[:], w2f[:].rearrange("(h p) c -> p h c", p=64))
            WS2S = pp.tile([64, H, 2], f32, tag="ws2s")
            nc.scalar.dma_start(WS2S[:], ws2[:].rearrange("(h p) c -> p h c", p=64))

            grp = [list(range(NCORES))]
            NG = H // GRP  # head groups

            # ------- stage B (replicated h, streamed x) fused with group-A attention -------
            HBF = pp.tile([128, JT, H * HCOL], bf16, tag="hbf")
            ones_view = HBF[:].rearrange("p jt (h c) -> p jt h c", c=HCOL)[:, :, :, D:D + 1]
            nc.vector.memset(ones_view.opt(), 1.0)
            xt_r = xtb[:].rearrange("(kt p) n -> p kt n", p=128)
            CW = N // NCH  # chunk width in source nodes
            TPC = CW // 128

            X2T32 = [None] * H

            def emit_wm(g0, jt):
                # group A (fused with h production) keeps ScalarE free for the
                # h-copies: its k=3 head uses the normalized DVE-TS path; only
                # group B's k=3 head runs the Prelu+Exp path on ScalarE
                act_k = ACT_K if g0 == GRP else ()
                wsc = pw.tile([128, GRP, SH], bf16, tag="w", name=f"w_{g0}_{jt}")
                for k in range(GRP):
                    h = g0 + k
                    if k in act_k:
                        epre = ps_ep.tile([128, SH], f32, tag="epre", name=f"ep_{h}_{jt}")
                        nc.scalar.activation(epre[:], SIB[h][:], ACT.Prelu,
                                             bias=SJB[:, jt, h:h + 1], alpha=ALPHA)
                        nc.scalar.activation(wsc[:, k, :], epre[:], ACT.Exp)
                    else:
                        # k=2 GpSimd, k=1 GpSimd 3 of 4 j-tiles, k=0 DVE; the
                        # first 4 j-tiles stay on DVE (GpSimd's queue is still
                        # draining broadcasts during the ramp)
                        on_pool = jt >= 4 and ((k == 2) or (k == 1 and jt % 4 != 3))
                        eng = nc.gpsimd if on_pool else nc.vector
                        eng.tensor_scalar(wsc[:, k, :], GB1[h][:],
                                          F1[:, jt, h:h + 1], E1[:, jt, h:h + 1],
                                          AT.mult, AT.max)  # noqa: E501
                wm = pw.tile([128, GRP, SH], bf16, tag="wm", name=f"wm_{g0}_{jt}")
                a_rep = ADJ[:, jt, :].unsqueeze(1).to_broadcast([128, GRP, SH])
                nc.vector.tensor_tensor(wm[:], wsc[:], a_rep, AT.mult)
                return wm

            def emit_mms(g0, jt, psAs, wm):
                for k in range(GRP):
                    h = g0 + k
                    nc.tensor.matmul(psAs[k][:], HBF[:, jt, h * HCOL:(h + 1) * HCOL],
                                     wm[:, k, :], start=(jt == 0), stop=(jt == JT - 1))

            def emit_scores(g0, jt, psAs):
                emit_mms(g0, jt, psAs, emit_wm(g0, jt))

            def emit_epilogue(g0):
                # (column-chunking group B by m-tile to start stage D earlier
                # was tried and regressed: the 4x instruction count costs more
                # than the earlier gather start saves)
                mslices = [slice(0, SH)]
                for k in range(GRP):
                    h = g0 + k
                    psA = psAs[k]
                    rc = pe.tile([1, SH], f32, tag="rc", name=f"rc_{h}", bufs=3)
                    nc.vector.reciprocal(rc[:], psA[D:D + 1, :])
                    rb = pe.tile([64, SH], f32, tag="rb", name=f"rb_{h}", bufs=3)
                    nc.gpsimd.partition_broadcast(rb[:], rc[:])
                    z = pe.tile([64, SH], f32, tag="z", name=f"z_{h}", bufs=3)
                    u = pe.tile([64, SH], f32, tag="u", name=f"u_{h}")
                    v = pe.tile([64, SH], f32, tag="v", name=f"v_{h}")
                    x2t = pp.tile([64, SH], f32, tag=f"x2t32_{h}", name=f"x2t_{h}")
                    for sl in mslices:
                        nc.vector.tensor_mul(z[:, sl], psA[0:D, sl], rb[:, sl])
                        nc.scalar.activation(u[:, sl], z[:, sl], ACT.Relu, scale=-1.0)
                        nc.scalar.activation(v[:, sl], u[:, sl], ACT.Exp, scale=-1.0)
                        nc.vector.scalar_tensor_tensor(x2t[:, sl], v[:, sl], -1.0,
                                                       z[:, sl], AT.add, AT.max)
                    X2T32[h] = x2t

            # LOOP1: stage-B h production + group-A (heads 0..3) attention per j-tile
            psAs = [ps_acc.tile([HCOL, SH], f32, tag="acc", name=f"psA_0_{k}")
                    for k in range(GRP)]
            # adjacency streams in 4-jt (512KB) chunks from the sync queue,
            # staying a few j-tiles ahead of consumption
            adj_sched = {0: (2, 6), 2: (6, 10), 5: (10, 14), 8: (14, 18),
                         11: (18, 22), 14: (22, 26), 17: (26, 30), 20: (30, 32)}
            xcs = {}
            xcs[0] = xc0

            def emit_h(jt):
                ch, t = divmod(jt, TPC)
                if t == 0 and ch > 0:
                    xc = px.tile([128, KT, CW], bf16, tag="xc", name=f"xc_{ch}")
                    nc.sync.dma_start(xc[:], xt_r[:, :, ch * CW:(ch + 1) * CW])
                    xcs[ch] = xc
                ph = ps_acc.tile([128, H * D], f32, tag="acc", name=f"ph_{jt}")
                for kt in range(KT):
                    nc.tensor.matmul(ph[:], xcs[ch][:, kt, t * 128:(t + 1) * 128],
                                     W1B[:, kt, :], start=(kt == 0), stop=(kt == KT - 1))
                dst = HBF[:, jt, :].rearrange("p (h c) -> p h c", c=HCOL)[:, :, 0:D]
                nc.scalar.copy(dst, ph[:].rearrange("p (h d) -> p h d", d=D))

            # software-pipeline the h production PROLOG tiles ahead: the
            # in-order PE queue would otherwise stall at the first attention
            # matmul (~8us, gated by broadcast->TS->mask) with no banked work
            PROLOG = 2
            for jt in range(PROLOG):
                emit_h(jt)
            for jt in range(JT):
                if jt in adj_sched:
                    lo, hi = adj_sched[jt]
                    nc.sync.dma_start(ADJ[:, lo:hi, :], adj_r[:, lo:hi, :])
                wm1 = emit_wm(0, jt)
                emit_mms(0, jt, psAs, wm1)
                # pipelined h production AFTER this j-tile's attention matmuls
                # so an x-chunk stall can't block them in the in-order PE queue
                if jt + PROLOG < JT:
                    emit_h(jt + PROLOG)
            emit_epilogue(0)

            # LOOP2: group-B (heads 4..7) attention
            psAs = [ps_acc.tile([HCOL, SH], f32, tag="acc", name=f"psA_4_{k}")
                    for k in range(GRP)]
            for jt in range(JT):
                emit_scores(GRP, jt, psAs)
            emit_epilogue(GRP)

            # bridge the group-B epilogue window (> HAM MID threshold) so
            # TensorE stays at 2.4 GHz into stage D
            warm1 = ps_acc.tile([HCOL, SH], f32, tag="acc", name="warm1")
            for wi in range(16):
                nc.tensor.matmul(warm1[:], HBF[:, wi % JT, 0:HCOL],
                                 HBF[:, wi % JT, 0:SH], start=True, stop=True)

            # ---------------- stage D: layer-2 shard compute (fp32) ----------------
            # single gather payload: [h2 as bf16 (40) | E2=exp(sj2) | F2=exp(0.2*sj2)]
            HB2S = pp.tile([128, MT, C + 2], bf16, tag="hb2s")
            for m in range(MT):
                ph2 = ps_acc.tile([128, C], f32, tag="acc", name=f"ph2_{m}")
                for h in range(H):
                    nc.tensor.matmul(ph2[:], X2T32[h][:, m * 128:(m + 1) * 128],
                                     W2BS[:, h, :], start=(h == 0), stop=(h == H - 1))
                nc.scalar.copy(HB2S[:, m, 0:C], ph2[:])
                psj2 = ps_acc.tile([128, 2], f32, tag="acc", name=f"psj2_{m}")
                for h in range(H):
                    nc.tensor.matmul(psj2[:], X2T32[h][:, m * 128:(m + 1) * 128],
                                     WS2S[:, h, :], start=(h == 0), stop=(h == H - 1))
                nc.scalar.activation(HB2S[:, m, C:C + 1], psj2[:, 0:1], ACT.Exp)
                nc.scalar.activation(HB2S[:, m, C + 1:C + 2], psj2[:, 0:1], ACT.Exp,
                                     scale=ALPHA)
            psi2 = ps_acc.tile([1, SH], f32, tag="acc")
            for h in range(H):
                nc.tensor.matmul(psi2[:], WS2S[:, h, 1:2], X2T32[h][:],
                                 start=(h == 0), stop=(h == H - 1))
            g2 = pe.tile([1, SH], bf16, tag="grow")
            nc.scalar.activation(g2[:], psi2[:], ACT.Exp, scale=-0.8)
            GB2 = pb.tile([128, SH], bf16, tag="gb2")
            nc.gpsimd.partition_broadcast(GB2[:], g2[:])

            hb2_bounce = pd.tile([SH, C + 2], bf16, tag="hb2_bounce")
            nc.sync.dma_start(hb2_bounce[:].rearrange("(m p) c -> p m c", p=128), HB2S[:])
            hb2f_d = nc.dram_tensor("hb2f_d", [N, C + 2], bf16, kind="Internal",
                                    addr_space="Shared")
            nc.gpsimd.collective_compute("AllGather", AT.bypass, replica_groups=grp,
                                         ins=[hb2_bounce.opt()], outs=[hb2f_d[:]])
            # (no warm matmuls here: they have no data dep on the collective,
            # so they fire during its early window and only delay stage E;
            # stage E itself is DVE/Pool-bound, so the PE p-state is moot)
            # one contiguous load of the gathered [h2|E2|F2] rows; the strided
            # sub-loads would each be descriptor-bound (~1.8us apiece), so
            # split on-chip with cheap 4x-mode DVE copies instead
            hb2f_r = hb2f_d[:].rearrange("(jt p) c -> p jt c", p=128)
            HB2A = pp.tile([128, JT, C + 2], bf16, tag="hb2a")
            nc.sync.dma_start(HB2A[:], hb2f_r[:])
            EF2 = pp.tile([128, JT, 2], f32, tag="ef2")
            nc.vector.tensor_copy(EF2[:], HB2A[:, :, C:C + 2])

            # [h2(40) | zero pad | ones at col 64] so the denominator row lands
            # on the 32-aligned psum partition 64
            HB2F = pp.tile([128, JT, HCOL], bf16, tag="hb2f")
            nc.vector.memset(HB2F[:], 0.0)
            nc.vector.tensor_copy(HB2F[:, :, 0:C], HB2A[:, :, 0:C])
            nc.vector.memset(HB2F[:, :, D:D + 1], 1.0)

            # ---------------- stage E: layer-2 attention (4-jt batches) ----------------
            ps2 = ps_acc.tile([HCOL, SH], f32, tag="acc")
            for jb in range(JT // 4):
                w2t = pw.tile([128, 4, SH], bf16, tag="w", name=f"w2t_{jb}")
                for t in range(4):
                    jt = jb * 4 + t
                    eng2 = nc.gpsimd if t >= 2 else nc.vector
                    eng2.tensor_scalar(w2t[:, t, :], GB2[:],
                                       EF2[:, jt, 1:2], EF2[:, jt, 0:1],
                                       AT.mult, AT.max)
                wm2 = pw.tile([128, 4, SH], bf16, tag="wm", name=f"wm2_{jb}")
                nc.vector.tensor_tensor(wm2[:], w2t[:], ADJ[:, jb * 4:(jb + 1) * 4, :], AT.mult)
                for t in range(4):
                    jt = jb * 4 + t
                    nc.tensor.matmul(ps2[:], HB2F[:, jt, :], wm2[:, t, :],
                                     start=(jt == 0), stop=(jt == JT - 1))
            rc2 = pe.tile([1, SH], f32, tag="rc", bufs=3)
            nc.vector.reciprocal(rc2[:], ps2[D:D + 1, :])
            rb2 = pe.tile([64, SH], f32, tag="rb", bufs=3)
            nc.gpsimd.partition_broadcast(rb2[:], rc2[:])
            OT = pe.tile([64, SH], f32, tag="z", bufs=3)
            nc.vector.tensor_mul(OT[:], ps2[0:64, :], rb2[:])
            nc.sync.dma_start(outT[:], OT[0:C, :])

    nc.compile()
    _BUILT["nc"] = nc
    return nc


def kernel(x, adj, W1, a1_src, a1_dst, W2, a2_src, a2_dst):
    global LAST_RESULTS
    from concourse.bass_utils import run_bass_kernel_spmd

    bf = ml_dtypes.bfloat16
    x = np.asarray(x, np.float32)
    adj = np.asarray(adj)
    W1 = np.asarray(W1, np.float32)
    W2 = np.asarray(W2, np.float32)
    a1_src = np.asarray(a1_src, np.float32)
    a1_dst = np.asarray(a1_dst, np.float32)
    a2_src = np.asarray(a2_src, np.float32)
    a2_dst = np.asarray(a2_dst, np.float32)

    xt = x.T.astype(bf)                                 # [F, N] (astype -> contiguous)
    adjt = adj.T.astype(bf)                             # [N(j), N(i)]
    w1b = np.ascontiguousarray(W1.transpose(1, 0, 2).reshape(F, H * D)).astype(bf)
    w2f = W2.astype(np.float32)
    ws2 = np.ascontiguousarray(np.stack([W2 @ a2_src, W2 @ a2_dst], axis=1)).astype(np.float32)

    # host-side linear projections for layer-1 scores (exact fp32)
    sj = x @ np.einsum("hfd,hd->fh", W1, a1_src)        # [N, H]
    si = x @ np.einsum("hfd,hd->fh", W1, a1_dst)        # [N, H]
    dev = lambda a: np.ascontiguousarray(
        a.reshape(JT, 128, H).transpose(1, 0, 2).reshape(128, JT * H)).astype(np.float32)
    E = np.exp(sj)
    Fj = np.exp(ALPHA * sj)

    e1f, f1f, sjb = dev(E), dev(Fj), dev(sj)
    nc = _build()
    in_maps = []
    for c in range(NCORES):
        lo, hi = c * SH, (c + 1) * SH
        si_own = si[lo:hi, :]                           # [SH, H]
        in_maps.append(dict(
            xtb=xt,
            adjt=np.ascontiguousarray(adjt[:, lo:hi]),
            w1b=w1b, w2f=w2f, ws2=ws2,
            e1f=e1f, f1f=f1f, sjb=sjb,
            g1r=np.ascontiguousarray(np.exp(-0.8 * si_own.T)).astype(bf),
            si1r=np.ascontiguousarray(si_own.T).astype(np.float32),
        ))
    res = run_bass_kernel_spmd(nc, in_maps, core_ids=list(range(NCORES)))
    LAST_RESULTS = res
    out = np.concatenate([res.results[c]["outT"].T for c in range(NCORES)], axis=0)
    return np.ascontiguousarray(out.astype(np.float32))

